# revision 1
# baseline (speedup 1.0000x reference)
import numpy as np

DT = 1.0 / 120.0
G = 9.81
K_SIGN = 100.0
TWO_PI = 2.0 * np.pi

N_CORES = 8


def _ekf_shard_loss(params, covariance_params, init_state, measurements):
    """Per-shard EKF loss SUM over this shard's segments (jax, neuron-friendly:
    closed-form 3x3 inverse/det instead of linalg ops)."""
    import jax.numpy as jnp
    from jax import lax

    dyna = jnp.abs(params)
    fric, damp = dyna[0], dyna[1]

    cp = covariance_params
    r0, r1, r2 = jnp.exp(cp[0]), jnp.exp(cp[1]), jnp.exp(cp[2])
    Q = jnp.diag(jnp.exp(jnp.stack([cp[3], cp[3], cp[4], cp[4], cp[5], cp[6]])))

    n = init_state.shape[0]
    P0 = jnp.broadcast_to(jnp.eye(6, dtype=init_state.dtype) * 0.01, (n, 6, 6))
    I6 = jnp.eye(6, dtype=init_state.dtype)
    a55 = 1.0 - DT * damp

    def step(carry, z):
        x, P = carry
        pos, vel, th, om = x[:, 0:2], x[:, 2:4], x[:, 4:5], x[:, 5:6]
        t = jnp.tanh(K_SIGN * vel)
        x_pred = jnp.concatenate(
            [pos + DT * vel,
             vel - DT * (damp * vel + fric * G * t),
             th + DT * om,
             om - DT * damp * om], axis=1)

        dv = 1.0 - DT * (damp + fric * G * K_SIGN * (1.0 - t * t))  # [n,2]
        F = jnp.broadcast_to(I6, (n, 6, 6))
        F = F.at[:, 0, 2].set(DT).at[:, 1, 3].set(DT).at[:, 4, 5].set(DT)
        F = F.at[:, 2, 2].set(dv[:, 0]).at[:, 3, 3].set(dv[:, 1])
        F = F.at[:, 5, 5].set(a55)
        P_pred = jnp.einsum('nij,njk,nlk->nil', F, P, F) + Q

        y0 = z[:, 0] - x_pred[:, 0]
        y1 = z[:, 1] - x_pred[:, 1]
        ang = z[:, 2] - x_pred[:, 4]
        ang = jnp.where(ang > 1.5 * np.pi, ang - TWO_PI,
                        jnp.where(ang < -1.5 * np.pi, ang + TWO_PI, ang))

        # S = P_pred[[0,1,4]][:,[0,1,4]] + diag(r)
        s00 = P_pred[:, 0, 0] + r0
        s01 = P_pred[:, 0, 1]
        s02 = P_pred[:, 0, 4]
        s11 = P_pred[:, 1, 1] + r1
        s12 = P_pred[:, 1, 4]
        s22 = P_pred[:, 4, 4] + r2

        c00 = s11 * s22 - s12 * s12
        c01 = s02 * s12 - s01 * s22
        c02 = s01 * s12 - s02 * s11
        c11 = s00 * s22 - s02 * s02
        c12 = s01 * s02 - s00 * s12
        c22 = s00 * s11 - s01 * s01
        det = s00 * c00 + s01 * c01 + s02 * c02
        rec = 1.0 / det
        # Sinv rows (symmetric)
        i00, i01, i02 = c00 * rec, c01 * rec, c02 * rec
        i11, i12, i22 = c11 * rec, c12 * rec, c22 * rec

        # w = Sinv @ y  [n,3]
        w0 = i00 * y0 + i01 * y1 + i02 * ang
        w1 = i01 * y0 + i11 * y1 + i12 * ang
        w2 = i02 * y0 + i12 * y1 + i22 * ang

        # Pc = P_pred[:, :, [0,1,4]]  [n,6,3]
        Pc = jnp.stack([P_pred[:, :, 0], P_pred[:, :, 1], P_pred[:, :, 4]], axis=2)
        # x_new = x_pred + Pc @ w
        w = jnp.stack([w0, w1, w2], axis=1)
        x_new = x_pred + jnp.einsum('nij,nj->ni', Pc, w)

        # K = Pc @ Sinv  [n,6,3]; P_new = P_pred - K @ Pc^T
        Sinv = jnp.stack([
            jnp.stack([i00, i01, i02], axis=1),
            jnp.stack([i01, i11, i12], axis=1),
            jnp.stack([i02, i12, i22], axis=1)], axis=1)
        K = jnp.einsum('nij,njk->nik', Pc, Sinv)
        P_new = P_pred - jnp.einsum('nij,nkj->nik', K, Pc)

        maha = y0 * w0 + y1 * w1 + ang * w2
        loss_t = 0.5 * jnp.sum(jnp.log(det) + maha)
        return (x_new, P_new), loss_t

    (_, _), losses = lax.scan(step, (init_state, P0),
                              jnp.transpose(measurements, (1, 0, 2)))
    return jnp.sum(losses)


def _ekf_numpy(params, covariance_params, init_state, measurements):
    """Pure-numpy reference-equivalent fallback (vectorized over N)."""
    dyna = np.abs(params).astype(np.float32)
    fric, damp = dyna[0], dyna[1]
    cp = covariance_params
    R = np.diag(np.exp(cp[:3])).astype(np.float32)
    Q = np.diag(np.exp(np.stack([cp[3], cp[3], cp[4], cp[4], cp[5], cp[6]]))).astype(np.float32)
    N = init_state.shape[0]
    midx = [0, 1, 4]
    x = init_state.copy()
    P = np.broadcast_to(np.eye(6, dtype=np.float32) * 0.01, (N, 6, 6)).copy()
    I6 = np.eye(6, dtype=np.float32)
    total = np.float64(0.0)
    for ti in range(measurements.shape[1]):
        z = measurements[:, ti, :]
        vel = x[:, 2:4]
        t = np.tanh(K_SIGN * vel)
        x_pred = np.concatenate(
            [x[:, 0:2] + DT * vel,
             vel - DT * (damp * vel + fric * G * t),
             x[:, 4:5] + DT * x[:, 5:6],
             x[:, 5:6] - DT * damp * x[:, 5:6]], axis=1).astype(np.float32)
        dv = 1.0 - DT * (damp + fric * G * K_SIGN * (1.0 - t * t))
        F = np.broadcast_to(I6, (N, 6, 6)).copy()
        F[:, 0, 2] = DT; F[:, 1, 3] = DT; F[:, 4, 5] = DT
        F[:, 2, 2] = dv[:, 0]; F[:, 3, 3] = dv[:, 1]
        F[:, 5, 5] = 1.0 - DT * damp
        P_pred = np.einsum('nij,njk,nlk->nil', F, P, F) + Q
        y = z - x_pred[:, midx]
        ang = y[:, 2]
        ang = np.where(ang > 1.5 * np.pi, ang - TWO_PI,
                       np.where(ang < -1.5 * np.pi, ang + TWO_PI, ang))
        y[:, 2] = ang
        S = P_pred[:, midx][:, :, midx] + R
        Sinv = np.linalg.inv(S.astype(np.float64)).astype(np.float32)
        K = np.einsum('nij,njk->nik', P_pred[:, :, midx], Sinv)
        x = x_pred + np.einsum('nij,nj->ni', K, y)
        KH = np.zeros((N, 6, 6), np.float32)
        KH[:, :, midx] = K
        P = np.einsum('nij,njk->nik', I6 - KH, P_pred)
        sign, logdet = np.linalg.slogdet(S.astype(np.float64))
        maha = np.einsum('ni,nij,nj->n', y, Sinv, y)
        total += 0.5 * np.sum(logdet + maha)
    return total


def kernel(params, covariance_params, init_state, measurements):
    """Shards the N segment dimension across the 8 NeuronCores (pure data
    parallel), params replicated; per-shard loss sums combined on host."""
    params = np.asarray(params, dtype=np.float32)
    covariance_params = np.asarray(covariance_params, dtype=np.float32)
    init_state = np.asarray(init_state, dtype=np.float32)
    measurements = np.asarray(measurements, dtype=np.float32)
    N = init_state.shape[0]

    try:
        import jax

        devs = jax.devices()
        n_sh = N_CORES if (len(devs) >= N_CORES and N % N_CORES == 0) else 1
        if n_sh > 1:
            shard_n = N // n_sh
            init_sh = init_state.reshape(n_sh, shard_n, 6)
            meas_sh = measurements.reshape(n_sh, shard_n, *measurements.shape[1:])
            pfun = jax.pmap(_ekf_shard_loss, axis_name='i',
                            in_axes=(None, None, 0, 0), devices=devs[:n_sh])
            sums = np.asarray(pfun(params, covariance_params, init_sh, meas_sh))
            if not np.all(np.isfinite(sums)):
                raise RuntimeError("non-finite shard sums")
            total = np.sum(sums.astype(np.float64))
        else:
            total = float(jax.jit(_ekf_shard_loss)(
                params, covariance_params, init_state, measurements))
        return np.float32(total / N)
    except Exception:
        return np.float32(_ekf_numpy(params, covariance_params,
                                     init_state, measurements) / N)

